# revision 1
# baseline (speedup 1.0000x reference)
# Trainium2 Bass kernel for the AttnBlock problem:
#   y = x + proj( attn( groupnorm(x) ) ),  single-head attention over H*W
#   positions, per batch element.  B=4, C=512, H=W=64 (N=4096), f32.
#
# Sharding: 8 cores = 4 batch elements x 2 query-halves.  Each core gets its
# batch's full (C, N) image with the spatial axis rotated so that its 2048
# query positions are local columns [0, 2048).  Attention is invariant to a
# permutation of the key set, and GroupNorm stats are permutation invariant,
# so every core runs an identical (SPMD) program.
#
# Device program per core:
#   GN:    bn_stats per 128-channel chunk + group (16-ch) aggregation via two
#          tiny PE matmuls with indicator matrices; per-channel affine a*x+d
#          applied on ScalarE -> h (bf16).
#   QKV:   k[o,n], q[o,i] (with bias, i = local 2048 queries), vT[n,o] in
#          bf16 via PE.  k-bias is dropped (softmax is invariant to it),
#          v-bias is folded into an effective proj bias  bp_eff = bp + wp@bv
#          computed on device.
#   Attn:  scores computed transposed  sT[j,i] = sum_c k[c,j] q[c,i] so the
#          softmax sum over keys j is a ones-matmul on PE; exp on ScalarE
#          (no max subtraction: |s| <= ~8 for this problem); PV contracts
#          over j:  o[c,i] = sum_j vT[j,c] pT[j,i]; normalization by 1/l via
#          a K=1 broadcast matmul + VectorE multiply.
#   Out:   proj matmul + bp_eff + residual x, f32, DMA to HBM.
import numpy as np
import ml_dtypes

B, C, H, W = 4, 512, 64, 64
N = H * W            # 4096 spatial positions
QH = N // 2          # 2048 queries per core
CH = C // 128        # 4 channel chunks
NJ = N // 128        # 32 key chunks
NI = QH // 512       # 4 query column blocks
EPS = 1e-6
SCALE = float(C) ** -0.5
NCORES = 8

_CACHE = {}


def _build_module():
    import concourse.bacc as bacc
    import concourse.bass as bass
    import concourse.tile as tile
    from concourse import mybir
    from contextlib import ExitStack

    f32 = mybir.dt.float32
    bf16 = mybir.dt.bfloat16
    AF = mybir.ActivationFunctionType
    OP = mybir.AluOpType

    # Bacc (not plain Bass): its compile() runs generate_event_semaphores /
    # move_matmul_waits_to_ldweights, which enforce the TRN2 one-wait-per-
    # instruction constraint that walrus codegen rejects otherwise.
    nc = bacc.Bacc("TRN2", num_devices=NCORES, enable_asserts=False)

    x_d = nc.dram_tensor("x", [C, N], f32, kind="ExternalInput").ap()
    wqT_d = nc.dram_tensor("wqT", [128, CH, C], bf16, kind="ExternalInput").ap()
    wkT_d = nc.dram_tensor("wkT", [128, CH, C], bf16, kind="ExternalInput").ap()
    wvT_d = nc.dram_tensor("wvT", [128, CH, C], bf16, kind="ExternalInput").ap()
    wpT_d = nc.dram_tensor("wpT", [128, CH, C], bf16, kind="ExternalInput").ap()
    bq_d = nc.dram_tensor("bq", [128, CH], f32, kind="ExternalInput").ap()
    bv_d = nc.dram_tensor("bv", [128, CH], f32, kind="ExternalInput").ap()
    bp_d = nc.dram_tensor("bp", [128, CH], f32, kind="ExternalInput").ap()
    gns_d = nc.dram_tensor("gns", [128, CH], f32, kind="ExternalInput").ap()
    gnb_d = nc.dram_tensor("gnb", [128, CH], f32, kind="ExternalInput").ap()
    ind16_d = nc.dram_tensor("ind16", [128, 8], f32, kind="ExternalInput").ap()
    indT_d = nc.dram_tensor("indT", [8, 128], f32, kind="ExternalInput").ap()
    onecf_d = nc.dram_tensor("onecf", [128, 1], f32, kind="ExternalInput").ap()
    y_d = nc.dram_tensor("y", [C, QH], f32, kind="ExternalOutput").ap()

    with tile.TileContext(nc) as tc, ExitStack() as ctx:
        consts = ctx.enter_context(tc.tile_pool(name="consts", bufs=1))
        persist = ctx.enter_context(tc.tile_pool(name="persist", bufs=1))

        wpT_sb = consts.tile([128, CH, C], bf16, name="wpT_sb")
        nc.sync.dma_start(wpT_sb, wpT_d)
        bq_sb = consts.tile([128, CH], f32, name="bq_sb")
        nc.sync.dma_start(bq_sb, bq_d)
        bv_sb = consts.tile([128, CH], f32, name="bv_sb")
        nc.sync.dma_start(bv_sb, bv_d)
        bp_sb = consts.tile([128, CH], f32, name="bp_sb")
        nc.sync.dma_start(bp_sb, bp_d)
        gns_sb = consts.tile([128, CH], f32, name="gns_sb")
        nc.sync.dma_start(gns_sb, gns_d)
        gnb_sb = consts.tile([128, CH], f32, name="gnb_sb")
        nc.sync.dma_start(gnb_sb, gnb_d)
        ind16_sb = consts.tile([128, 8], f32, name="ind16_sb")
        nc.sync.dma_start(ind16_sb, ind16_d)
        indT_sb = consts.tile([8, 128], f32, name="indT_sb")
        nc.sync.dma_start(indT_sb, indT_d)
        onecf_sb = consts.tile([128, 1], f32, name="onecf_sb")
        nc.sync.dma_start(onecf_sb, onecf_d)
        bv_bf = consts.tile([128, CH], bf16, name="bv_bf")
        # cast on GpSimd so the early bp_eff matmuls don't wait behind the
        # GN stats prefix in the VectorE stream
        nc.gpsimd.tensor_copy(bv_bf, bv_sb)

        k_big = persist.tile([128, CH, N], bf16, name="k_big")
        v_big = persist.tile([128, NJ, C], bf16, name="v_big")
        q_big = persist.tile([128, CH, QH], bf16, name="q_big")
        bpe_sb = persist.tile([128, CH], f32, name="bpe_sb")

        # ---------------- Phase 1: GroupNorm + QKV ----------------
        # 1a computes per-channel GN affine (a, d) from bn_stats; 1b applies
        # the affine slice-by-slice and feeds QKV matmuls immediately, so PE
        # starts ~25us in instead of waiting for the whole GN pass.
        with tc.tile_pool(name="xp", bufs=1) as xp, \
                tc.tile_pool(name="hp", bufs=3) as hp, \
                tc.tile_pool(name="wts", bufs=1) as wts, \
                tc.tile_pool(name="gt", bufs=2) as gt, \
                tc.tile_pool(name="pqkv", bufs=3, space="PSUM") as pqkv, \
                tc.tile_pool(name="psml", bufs=3, space="PSUM") as psml:

            # 1a: x chunks stay resident; stats per 512-col slice as the DMA
            # lands; all 4 chunks' group aggregation batched into one matmul
            # pair + one tiny-op chain.  (Weight DMAs are emitted after the
            # x DMAs — x is on the critical path, weights are not.)
            ad_all = gt.tile([128, CH, 2], f32, name="ad_all")
            x_ts = []
            for cc in range(CH):
                with nc.named_scope(f"gn{cc}"):
                    x_t = xp.tile([128, N], f32, name=f"x_t{cc}")
                    x_ts.append(x_t)
                    xv = x_t.rearrange("p (s f) -> p s f", f=512)
                    stats = gt.tile([128, 8, 6], f32, name="stats")
                    for s in range(8):
                        nc.sync.dma_start(
                            xv[:, s, :],
                            x_d[cc * 128:(cc + 1) * 128,
                                s * 512:(s + 1) * 512])
                        nc.vector.bn_stats(stats[:, s, :], xv[:, s, :])
                    mv = gt.tile([128, 2], f32, name="mv")
                    nc.vector.bn_aggr(mv, stats)
                    # per-channel (mean, mean^2 + var)
                    cm = gt.tile([128, 2], f32, name="cm")
                    nc.vector.tensor_copy(cm[:, 0:1], mv[:, 0:1])
                    nc.vector.scalar_tensor_tensor(
                        out=cm[:, 1:2], in0=mv[:, 0:1], scalar=mv[:, 0:1],
                        in1=mv[:, 1:2], op0=OP.mult, op1=OP.add)
                    # per-chunk group aggregate (16-ch groups sit inside one
                    # chunk) so each chunk's chain overlaps later stats
                    gs_ps = psml.tile([8, 2], f32, name="gs_ps", tag="sm")
                    nc.tensor.matmul(gs_ps, lhsT=ind16_sb, rhs=cm,
                                     start=True, stop=True)
                    gs = gt.tile([8, 2], f32, name="gs")
                    nc.vector.tensor_copy(gs, gs_ps)
                    gv = gt.tile([8, 4], f32, name="gv")
                    nc.vector.scalar_tensor_tensor(
                        out=gv[:, 0:1], in0=gs[:, 0:1], scalar=gs[:, 0:1],
                        in1=gs[:, 1:2], op0=OP.mult, op1=OP.subtract)
                    nc.vector.tensor_scalar(
                        out=gv[:, 0:1], in0=gv[:, 0:1], scalar1=-1.0,
                        scalar2=EPS, op0=OP.mult, op1=OP.add)
                    # rstd = 1/sqrt(var+eps), one Newton refinement
                    nc.scalar.activation(gv[:, 1:2], gv[:, 0:1], AF.Sqrt)
                    nc.vector.reciprocal(gv[:, 2:3], gv[:, 1:2])
                    nc.vector.tensor_mul(gv[:, 3:4], gv[:, 2:3], gv[:, 2:3])
                    nc.vector.tensor_mul(gv[:, 3:4], gv[:, 3:4], gv[:, 0:1])
                    nc.vector.tensor_scalar(
                        out=gv[:, 3:4], in0=gv[:, 3:4], scalar1=-0.5,
                        scalar2=1.5, op0=OP.mult, op1=OP.add)
                    nc.vector.tensor_mul(gs[:, 1:2], gv[:, 2:3], gv[:, 3:4])
                    # broadcast (gmean, rstd) back to channels
                    mr_ps = psml.tile([128, 2], f32, name="mr_ps", tag="sm")
                    nc.tensor.matmul(mr_ps, lhsT=indT_sb, rhs=gs,
                                     start=True, stop=True)
                    ad = ad_all[:, cc, :]
                    nc.vector.tensor_mul(ad[:, 0:1], mr_ps[:, 1:2],
                                         gns_sb[:, cc:cc + 1])
                    nc.vector.tensor_mul(ad[:, 1:2], mr_ps[:, 0:1],
                                         ad[:, 0:1])
                    nc.vector.tensor_sub(ad[:, 1:2], gnb_sb[:, cc:cc + 1],
                                         ad[:, 1:2])
            wqT_sb = wts.tile([128, CH, C], bf16, name="wqT_sb")
            nc.sync.dma_start(wqT_sb, wqT_d)
            wkT_sb = wts.tile([128, CH, C], bf16, name="wkT_sb")
            nc.sync.dma_start(wkT_sb, wkT_d)
            wvT_sb = wts.tile([128, CH, C], bf16, name="wvT_sb")
            nc.sync.dma_start(wvT_sb, wvT_d)

            # 1b: per 512-column slice: GN apply -> k / q / vT matmuls
            for n5 in range(N // 512):
                with nc.named_scope(f"qkv{n5}"):
                    h_sl = hp.tile([128, CH, 512], bf16, name="h_sl")
                    for cc in range(CH):
                        nc.scalar.activation(
                            h_sl[:, cc, :],
                            x_ts[cc][:, n5 * 512:(n5 + 1) * 512],
                            AF.Identity,
                            bias=ad_all[:, cc, 1:2], scale=ad_all[:, cc, 0:1])
                    if n5 < NI:  # q for local queries, with bias
                        for oc in range(CH):
                            q_ps = pqkv.tile([128, 512], f32, name="q_ps",
                                             tag="mm")
                            for cc in range(CH):
                                nc.tensor.matmul(
                                    q_ps,
                                    lhsT=wqT_sb[:, cc, oc * 128:(oc + 1) * 128],
                                    rhs=h_sl[:, cc, :],
                                    start=(cc == 0), stop=(cc == CH - 1))
                            nc.scalar.activation(
                                q_big[:, oc, n5 * 512:(n5 + 1) * 512], q_ps,
                                AF.Identity, bias=bq_sb[:, oc:oc + 1])
                    for oc in range(CH):  # k, no bias (softmax-invariant)
                        k_ps = pqkv.tile([128, 512], f32, name="k_ps",
                                         tag="mm")
                        for cc in range(CH):
                            nc.tensor.matmul(
                                k_ps,
                                lhsT=wkT_sb[:, cc, oc * 128:(oc + 1) * 128],
                                rhs=h_sl[:, cc, :],
                                start=(cc == 0), stop=(cc == CH - 1))
                        nc.vector.tensor_copy(
                            k_big[:, oc, n5 * 512:(n5 + 1) * 512], k_ps)
                    for j4 in range(4):  # vT (v-bias folded into bp_eff)
                        jn = n5 * 4 + j4
                        v_ps = pqkv.tile([128, 512], f32, name="v_ps",
                                         tag="mm")
                        for cc in range(CH):
                            nc.tensor.matmul(
                                v_ps,
                                lhsT=h_sl[:, cc, j4 * 128:(j4 + 1) * 128],
                                rhs=wvT_sb[:, cc, :],
                                start=(cc == 0), stop=(cc == CH - 1))
                        nc.vector.tensor_copy(v_big[:, jn, :], v_ps)

            with nc.named_scope("bpe"):
                # bp_eff = bp + wp @ bv
                for oc in range(CH):
                    bpe_ps = psml.tile([128, 1], f32, name="bpe_ps", tag="sm")
                    for cc in range(CH):
                        nc.tensor.matmul(
                            bpe_ps,
                            lhsT=wpT_sb[:, cc, oc * 128:(oc + 1) * 128],
                            rhs=bv_bf[:, cc:cc + 1],
                            start=(cc == 0), stop=(cc == CH - 1))
                    nc.vector.tensor_add(bpe_sb[:, oc:oc + 1], bpe_ps,
                                         bp_sb[:, oc:oc + 1])

        # ---------------- Phase 2: attention + proj + residual ----------------
        with tc.tile_pool(name="pp", bufs=2) as pp, \
                tc.tile_pool(name="op", bufs=2) as op_, \
                tc.tile_pool(name="asml", bufs=3) as asml, \
                tc.tile_pool(name="yp", bufs=3) as yp, \
                tc.tile_pool(name="pss", bufs=3, space="PSUM") as pss, \
                tc.tile_pool(name="psl", bufs=2, space="PSUM") as psl, \
                tc.tile_pool(name="pspv", bufs=2, space="PSUM") as pspv, \
                tc.tile_pool(name="drp", bufs=2, space="DRAM") as drp:
            for ic in range(NI):
                with nc.named_scope(f"attn{ic}"):
                    p_big = pp.tile([128, NJ, 512], bf16, name="p_big")
                    # l accumulated per-partition on VectorE (cheap), then one
                    # f32 ones-matmul folds the 128 partial rows on PE.
                    l_acc = asml.tile([128, 512], f32, name="l_acc")
                    for jc in range(NJ):
                        s_ps = pss.tile([128, 512], f32, name="s_ps")
                        for cc in range(CH):
                            nc.tensor.matmul(
                                s_ps,
                                lhsT=k_big[:, cc, jc * 128:(jc + 1) * 128],
                                rhs=q_big[:, cc, ic * 512:(ic + 1) * 512],
                                start=(cc == 0), stop=(cc == CH - 1))
                        nc.scalar.activation(p_big[:, jc, :], s_ps, AF.Exp,
                                             scale=SCALE)
                        if jc == 0:
                            nc.vector.tensor_copy(l_acc, p_big[:, jc, :])
                        else:
                            nc.vector.tensor_add(l_acc, l_acc,
                                                 p_big[:, jc, :])
                    # PV first in the PE stream; the l fold + 1/l broadcast
                    # (which wait on the VectorE l_acc chain) are emitted
                    # after it so they never head-of-line block PV.
                    o_sb = op_.tile([128, CH, 512], bf16, name="o_sb")
                    pv_list = []
                    for cc in range(CH):
                        pv_ps = pspv.tile([128, 512], f32, name="pv_ps",
                                          tag="pv")
                        pv_list.append(pv_ps)
                        for jc in range(NJ):
                            nc.tensor.matmul(
                                pv_ps,
                                lhsT=v_big[:, jc, cc * 128:(cc + 1) * 128],
                                rhs=p_big[:, jc, :],
                                start=(jc == 0), stop=(jc == NJ - 1))
                        if cc == 0:
                            l_ps = psl.tile([1, 512], f32, name="l_ps")
                            nc.tensor.matmul(l_ps, lhsT=onecf_sb, rhs=l_acc,
                                             start=True, stop=True)
                            recip = asml.tile([1, 512], f32, name="recip")
                            nc.vector.reciprocal(recip, l_ps)
                            # broadcast 1/l across partitions via DRAM bounce
                            rd = drp.tile([1, 512], f32, name="rd")
                            nc.sync.dma_start(rd, recip)
                            rb = asml.tile([128, 512], f32, name="rb")
                            rd_b = bass.AP(
                                tensor=rd.tensor, offset=rd.offset,
                                ap=[[0, 128]] + list(rd.ap[1:]))
                            nc.sync.dma_start(rb, rd_b)
                        nc.vector.tensor_mul(o_sb[:, cc, :], pv_ps, rb)
                with nc.named_scope(f"out{ic}"):
                    for oc in range(CH):
                        pj_ps = pspv.tile([128, 512], f32, name="pj_ps",
                                          tag="pv")
                        for cc in range(CH):
                            nc.tensor.matmul(
                                pj_ps,
                                lhsT=wpT_sb[:, cc, oc * 128:(oc + 1) * 128],
                                rhs=o_sb[:, cc, :],
                                start=(cc == 0), stop=(cc == CH - 1))
                        xres = yp.tile([128, 512], f32, name="xres")
                        nc.sync.dma_start(
                            xres,
                            x_d[oc * 128:(oc + 1) * 128,
                                ic * 512:(ic + 1) * 512])
                        y_sb = yp.tile([128, 512], f32, name="y_sb")
                        nc.vector.scalar_tensor_tensor(
                            out=y_sb, in0=pj_ps, scalar=bpe_sb[:, oc:oc + 1],
                            in1=xres, op0=OP.add, op1=OP.add)
                        nc.sync.dma_start(
                            y_d[oc * 128:(oc + 1) * 128,
                                ic * 512:(ic + 1) * 512], y_sb)
    nc.compile()
    return nc


def get_module():
    if "nc" not in _CACHE:
        _CACHE["nc"] = _build_module()
    return _CACHE["nc"]


def _chunked_vec(v):
    # [C] -> [128, CH]: column k holds channels [128k, 128(k+1))
    return np.ascontiguousarray(np.asarray(v, np.float32).reshape(CH, 128).T)


def _wT_chunked(w):
    # [O, C] weight -> lhsT layout [128, CH, O]: [c_in_chunk, chunk, o]
    wT = np.asarray(w, np.float32).T.reshape(CH, 128, C).transpose(1, 0, 2)
    return np.ascontiguousarray(wT.astype(ml_dtypes.bfloat16))


def make_in_maps(inputs):
    x = np.asarray(inputs["x"], np.float32).reshape(B, C, N)
    ind16 = np.zeros((128, 8), np.float32)
    for c in range(128):
        ind16[c, c // 16] = 1.0 / 16.0
    indT = np.zeros((8, 128), np.float32)
    for c in range(128):
        indT[c // 16, c] = 1.0
    shared = {
        "wqT": _wT_chunked(inputs["wq"]),
        "wkT": _wT_chunked(inputs["wk"]),
        "wvT": _wT_chunked(inputs["wv"]),
        "wpT": _wT_chunked(inputs["wp"]),
        "bq": _chunked_vec(inputs["bq"]),
        "bv": _chunked_vec(inputs["bv"]),
        "bp": _chunked_vec(inputs["bp"]),
        "gns": _chunked_vec(inputs["gn_scale"]),
        "gnb": _chunked_vec(inputs["gn_bias"]),
        "ind16": ind16,
        "indT": indT,
        "onecf": np.ones((128, 1), np.float32),
    }
    in_maps = []
    for core in range(NCORES):
        b, half = divmod(core, 2)
        xb = x[b]
        if half:
            xl = np.ascontiguousarray(
                np.concatenate([xb[:, QH:], xb[:, :QH]], axis=1))
        else:
            xl = np.ascontiguousarray(xb)
        in_maps.append({"x": xl, **shared})
    return in_maps


def assemble(results, out_dtype=np.float32):
    y = np.empty((B, C, N), np.float32)
    for core in range(NCORES):
        b, half = divmod(core, 2)
        y[b, :, half * QH:(half + 1) * QH] = results[core]["y"]
    return y.reshape(B, C, H, W).astype(out_dtype, copy=False)


def _get_runner():
    """Build the jitted 8-core executable once per process (mirrors
    bass2jax.run_bass_via_pjrt's multi-core branch, without re-tracing
    on every call)."""
    if "runner" in _CACHE:
        return _CACHE["runner"]
    import jax
    from jax.sharding import Mesh, PartitionSpec
    import warnings
    with warnings.catch_warnings():
        warnings.simplefilter("ignore")
        from jax.experimental.shard_map import shard_map
    from concourse import bass2jax, mybir

    nc = get_module()
    bass2jax.install_neuronx_cc_hook()
    partition_name = (nc.partition_id_tensor.name
                      if nc.partition_id_tensor else None)
    in_names, out_names, out_avals = [], [], []
    for alloc in nc.m.functions[0].allocations:
        if not isinstance(alloc, mybir.MemoryLocationSet):
            continue
        name = alloc.memorylocations[0].name
        if alloc.kind == "ExternalInput":
            if name != partition_name:
                in_names.append(name)
        elif alloc.kind == "ExternalOutput":
            out_names.append(name)
            out_avals.append(jax.core.ShapedArray(
                tuple(alloc.tensor_shape), mybir.dt.np(alloc.dtype)))
    all_in_names = list(in_names) + out_names
    if partition_name:
        all_in_names.append(partition_name)

    def _body(*args):
        operands = list(args)
        if partition_name:
            operands.append(bass2jax.partition_id_tensor())
        return tuple(bass2jax._bass_exec_p.bind(
            *operands, out_avals=tuple(out_avals),
            in_names=tuple(all_in_names), out_names=tuple(out_names),
            lowering_input_output_aliases=(),
            sim_require_finite=True, sim_require_nnan=True, nc=nc))

    mesh = Mesh(np.asarray(jax.devices()[:NCORES]), ("core",))
    n_args = len(in_names) + len(out_names)
    fn = jax.jit(shard_map(_body, mesh=mesh,
                           in_specs=(PartitionSpec("core"),) * n_args,
                           out_specs=(PartitionSpec("core"),) * len(out_names),
                           check_rep=False),
                 keep_unused=True)
    zeros = [np.zeros((NCORES * av.shape[0], *av.shape[1:]), av.dtype)
             for av in out_avals]
    _CACHE["runner"] = (fn, in_names, out_names, out_avals, zeros)
    return _CACHE["runner"]


def kernel(**inputs):
    import jax

    fn, in_names, out_names, out_avals, zeros = _get_runner()
    in_maps = make_in_maps(inputs)
    concat = [np.concatenate([np.asarray(in_maps[c][k])
                              for c in range(NCORES)], axis=0)
              for k in in_names]
    outs = fn(*concat, *zeros)
    jax.block_until_ready(outs)
    yi = out_names.index("y")
    y_g = np.asarray(outs[yi]).reshape(NCORES, *out_avals[yi].shape)
    results = [{"y": y_g[c]} for c in range(NCORES)]
    return assemble(results, np.asarray(inputs["x"]).dtype)


if __name__ == "__main__":
    nc = get_module()
    print("module built ok")

